# revision 14
# baseline (speedup 1.0000x reference)
"""CRF loss (nn_CRFLoss) Trainium2 Bass kernel.

Math
----
reference:
    tg_energy = sum of scores.reshape(L,B,T*T)[l,b,target[l,b]] where mask
    alpha_0   = scores[0,:,START,:]                      # (B, T)
    alpha_l   = logsumexp_from(alpha_{l-1}[:,from,None] + scores[l])  (masked)
    loss      = (sum_b alpha_L[b, END] - tg_energy) / B

Device algorithm (per core, data-parallel over batch, B_loc = 16):
    Work in the exp domain with a per-batch running log-offset:
        V_0[b,f]  = exp(alpha_0[b,f] - m0_b)             (host-computed, tiny)
        S_l[b,t]  = sum_f V_{l-1}[b,f] * exp(scores_l[b,f,t])
    computed as out = Y_i^T @ W[:, 2i:2i+2] on PE (Y block as the stationary
    operand), which lands S already TRANSPOSED: ST[to, b] in PSUM.  The next
    step's block-diagonal weights W are then rebuilt with just two strided
    partition-crossing DVE copies (PSUM->SBUF, bf16 cast).  Every R steps S
    is divided by the (recorded) row ST[0, :] - any positive per-batch scale
    preserves the invariant alpha_l = m0 + sum(log recorded) + log S_l, and
    entries of S within one batch are within e^+-11 of each other, so this
    keeps everything in fp32/bf16 range.
    exp(scores) is computed by ACT in 8-step chunks (independent of the
    recurrence), scores stream from HBM in 2 MB contiguous DMAs.

Host does: batch sharding, a pure layout permutation of each core's score
shard so DMA reads are contiguous, the (tiny) initial V_0, gather offsets
for tg_energy, and the final scalar epilogue over 8 * (16, 64) outputs.
"""

import os
import numpy as np

import concourse.bass as bass
import concourse.bacc as bacc
import concourse.tile as tile
from concourse import mybir
from concourse.bass_utils import run_bass_kernel_spmd

# problem constants (hardcoded per harness contract)
L = 256
B = 128
T = 64
START_TAG = 62
END_TAG = 63

N_CORES = 8
B_LOC = B // N_CORES          # 16 batches per core
NTILES = B_LOC // 2           # 8 matmul tiles, 2 batches each
CK = 8                        # scan steps per DMA/exp chunk
RESCALE = 5                   # rescale period (applied 1 step late)

FP32 = mybir.dt.float32
BF16 = mybir.dt.bfloat16
I32 = mybir.dt.int32


def build_nc(l_steps=L, trace_label="crf", groups=2,
             ck=None):
    """Build the Bass kernel (see module docstring).  `groups` independent
    batch-group recurrences are interleaved so the serial PE->DVE->PE chain
    of one group hides under the other's engine work.

    Inputs (per core):
       xs   [l_steps, 128, 512] f32  : permuted scores,
                                       xs[l, 64h+f, 64i+t] = scores of batch
                                       2i+h  (f=from-tag, t=to-tag)
       w0   [128, 16] f32            : initial block-diag V_0 weights
                                       (col b = batch, rows 64*(b%2)+f)
       goff [128, ...] i32           : flat element indices into xs for the
                                       gold-path gather (tg_energy)
       gwgt [128, ...] f32           : mask weights for the gather
    Outputs:
       out_s  [64, 16] f32 : final-step ST (row = to, col = batch)
       out_mx [1, 1024] f32: recorded rescale divisors, col 16*k + b
                             (unused cols == 1.0)
       out_tg [128, 1] f32 : partial sums of gathered gold-path scores
    """
    nc = bacc.Bacc("TRN2", name=trace_label)

    xs = nc.dram_tensor("xs", [l_steps, 128, 512], FP32, kind="ExternalInput")
    w0 = nc.dram_tensor("w0", [128, B_LOC], FP32, kind="ExternalInput")
    goff = nc.dram_tensor("goff", [128, l_steps * B_LOC // 128], I32,
                          kind="ExternalInput")
    gwgt = nc.dram_tensor("gwgt", [128, l_steps * B_LOC // 128], FP32,
                          kind="ExternalInput")
    out_s = nc.dram_tensor("out_s", [T, B_LOC], FP32, kind="ExternalOutput")
    out_mx = nc.dram_tensor("out_mx", [1, 1024], FP32, kind="ExternalOutput")
    out_tg = nc.dram_tensor("out_tg", [128, 1], FP32, kind="ExternalOutput")

    if ck is None:
        ck = int(os.environ.get("CRF_CK", CK))
    n_gcols = l_steps * B_LOC // 128
    n_chunks = (l_steps + ck - 1) // ck
    ng = NTILES // groups          # tiles per group
    nb = 2 * ng                    # batches (= W/ST cols) per group

    with tile.TileContext(nc) as tc:
        with (
            tc.tile_pool(name="xpool", bufs=3) as xpool,
            tc.tile_pool(name="ypool", bufs=3) as ypool,
            tc.tile_pool(name="spool", bufs=2, space="PSUM") as spool,
            tc.tile_pool(name="rpool", bufs=1, space="PSUM") as rpool,
            tc.tile_pool(name="singles", bufs=1) as singles,
        ):
            # persistent per-group state
            wbuf, rcps, rcp_reps = [], [], []
            for g in range(groups):
                wbuf.append([
                    singles.tile([128, nb], BF16, tag=f"wA{g}",
                                 name=f"wA{g}"),
                    singles.tile([128, nb], BF16, tag=f"wB{g}",
                                 name=f"wB{g}")])
                rcps.append(singles.tile([1, nb], FP32, tag=f"rcp{g}",
                                         name=f"rcp{g}"))
                rcp_reps.append(singles.tile([128, nb], FP32,
                                             tag=f"rcp_rep{g}",
                                             name=f"rcp_rep{g}"))
            maxbuf = singles.tile([1, 1024], FP32, tag="maxbuf")
            w0sb = singles.tile([128, B_LOC], FP32, tag="w0sb")
            ones1 = singles.tile([1, 128], FP32, tag="ones1")
            nc.vector.memset(ones1[:], 1.0)

            nc.sync.dma_start(out=w0sb[:], in_=w0.ap())
            for g in range(groups):
                nc.vector.tensor_copy(out=wbuf[g][0][:],
                                      in_=w0sb[:, g * nb:(g + 1) * nb])
                nc.vector.memset(wbuf[g][1][:], 0.0)
            nc.vector.memset(maxbuf[:], 1.0)

            # ---- streaming exp(scores) ----
            ychunks = []
            for c in range(n_chunks):
                l0 = c * ck
                nsteps = min(ck, l_steps - l0)
                xc = xpool.tile([128, ck, 512], FP32, tag="xc")
                nc.sync.dma_start(out=xc[:, :nsteps, :],
                                  in_=xs.ap()[l0:l0 + nsteps, :, :]
                                  .rearrange("l p f -> p l f"))
                yc = ypool.tile([128, ck, 512], BF16, tag="yc")
                nc.scalar.activation(out=yc[:, :nsteps, :],
                                     in_=xc[:, :nsteps, :],
                                     func=mybir.ActivationFunctionType.Exp)
                ychunks.append(yc)

            # ---- main scan ----
            n_resc = 0
            pend_scale = [False] * groups
            sts = [None] * groups
            for l in range(1, l_steps):
                yc = ychunks[l // ck]
                l_sub = l % ck
                do_resc = (l % RESCALE == 0) and l != l_steps - 1
                for g in range(groups):
                    if os.environ.get("CRF_SKIP_MM"):
                        break
                    wr = wbuf[g][(l + 1) % 2]   # step 1 reads wbuf[g][0]
                    ww = wbuf[g][l % 2]
                    st = spool.tile([T, nb], FP32, tag=f"st{g}",
                                    name=f"st{g}")
                    sts[g] = st
                    for j in range(ng):
                        i = g * ng + j          # global tile / Y-block index
                        nc.tensor.matmul(
                            out=st[:, 2 * j:2 * j + 2],
                            lhsT=yc[:, l_sub, 64 * i:64 * i + 64],
                            rhs=wr[:, 2 * j:2 * j + 2],
                            start=True, stop=True,
                        )
                for g in range(groups):
                    if os.environ.get("CRF_SKIP_REBUILD"):
                        break
                    st, ww = sts[g], wbuf[g][l % 2]
                    # rebuild next-step block-diag weights (bf16 cast);
                    # a pending rescale folds in as a multiply.
                    if pend_scale[g]:
                        nc.vector.tensor_tensor(
                            out=ww[0:64, 0:nb:2], in0=st[0:64, 0:nb:2],
                            in1=rcp_reps[g][0:64, 0:nb:2],
                            op=mybir.AluOpType.mult)
                        nc.vector.tensor_tensor(
                            out=ww[64:128, 1:nb:2], in0=st[0:64, 1:nb:2],
                            in1=rcp_reps[g][64:128, 1:nb:2],
                            op=mybir.AluOpType.mult)
                        pend_scale[g] = False
                    else:
                        nc.vector.tensor_copy(out=ww[0:64, 0:nb:2],
                                              in_=st[0:64, 0:nb:2])
                        nc.vector.tensor_copy(out=ww[64:128, 1:nb:2],
                                              in_=st[0:64, 1:nb:2])
                if do_resc:
                    # record row 0 of each group's ST as the divisor; the
                    # reciprocal is folded into the NEXT step's rebuild (so
                    # this stays off the serial chain).  maxbuf layout:
                    # col 16*k + b  (b = global batch = g*nb + local col).
                    for g in range(groups):
                        st = sts[g]
                        c0 = B_LOC * n_resc + g * nb
                        nc.vector.tensor_copy(
                            out=maxbuf[0:1, c0:c0 + nb], in_=st[0:1, :])
                        nc.vector.reciprocal(out=rcps[g][:], in_=st[0:1, :])
                        # broadcast rcp across partitions with a K=1 matmul
                        # (keeps the gpsimd queue free for the gather DMA)
                        rps = rpool.tile([128, nb], FP32, tag=f"rps{g}",
                                         name=f"rps{g}")
                        nc.tensor.matmul(out=rps[:], lhsT=ones1[:],
                                         rhs=rcps[g][:],
                                         start=True, stop=True)
                        nc.vector.tensor_copy(out=rcp_reps[g][:], in_=rps[:])
                        pend_scale[g] = True
                    n_resc += 1

            # final-step ST and recorded divisors out
            sf = singles.tile([T, B_LOC], FP32, tag="sf")
            for g in range(groups):
                nc.vector.tensor_copy(out=sf[:, g * nb:(g + 1) * nb],
                                      in_=sts[g][:])
            nc.sync.dma_start(out=out_s.ap(), in_=sf[:])
            nc.sync.dma_start(out=out_mx.ap(), in_=maxbuf[:])

            # ---- gold-path gather (independent of the scan) ----
            goff_sb = singles.tile([128, n_gcols], I32, tag="goff")
            gwgt_sb = singles.tile([128, n_gcols], FP32, tag="gwgt")
            gbuf = singles.tile([128, n_gcols], FP32, tag="gbuf")
            tgp = singles.tile([128, 1], FP32, tag="tgp")
            nc.sync.dma_start(out=goff_sb[:], in_=goff.ap())
            nc.sync.dma_start(out=gwgt_sb[:], in_=gwgt.ap())
            n_elem = l_steps * 128 * 512
            xs_flat = bass.AP(tensor=xs, offset=0, ap=[[1, n_elem], [1, 1]])
            if not os.environ.get("CRF_SKIP_GATHER"):
                if os.environ.get("CRF_GATHER_SPLIT"):
                    for cidx in range(n_gcols):
                        nc.gpsimd.indirect_dma_start(
                            out=gbuf[:, cidx:cidx + 1],
                            out_offset=None,
                            in_=xs_flat,
                            in_offset=bass.IndirectOffsetOnAxis(
                                ap=goff_sb[:, cidx:cidx + 1], axis=0),
                        )
                else:
                    nc.gpsimd.indirect_dma_start(
                        out=gbuf[:],
                        out_offset=None,
                        in_=xs_flat,
                        in_offset=bass.IndirectOffsetOnAxis(
                            ap=goff_sb[:], axis=0),
                    )
            nc.vector.tensor_mul(out=gbuf[:], in0=gbuf[:], in1=gwgt_sb[:])
            nc.vector.reduce_sum(out=tgp[:], in_=gbuf[:],
                                 axis=mybir.AxisListType.X)
            nc.sync.dma_start(out=out_tg.ap(), in_=tgp[:])

    nc.finalize()
    return nc


def host_prepare(scores, target, mask, l_steps=L):
    """Shard + permute inputs for all cores. Returns (in_maps, m0_all)."""
    scores = np.asarray(scores, dtype=np.float32)
    target = np.asarray(target)
    mask = np.asarray(mask).astype(bool)

    in_maps = []
    m0_all = []
    for core in range(N_CORES):
        b0 = core * B_LOC
        sh = scores[:, b0:b0 + B_LOC]          # (L, 16, 64, 64)
        msk = mask[:, b0:b0 + B_LOC]           # (L, 16)
        tgt = target[:, b0:b0 + B_LOC, 0]      # (L, 16)

        if not msk[1:].all():
            # masked scan step == identity transition in the exp domain
            sh = sh.copy()
            ident = np.full((T, T), -1e9, dtype=np.float32)
            np.fill_diagonal(ident, 0.0)
            ls, bs = np.nonzero(~msk)
            for lz, bz in zip(ls, bs):
                if lz >= 1:
                    sh[lz, bz] = ident

        # initial state from step 0
        a0 = sh[0, :, START_TAG, :].astype(np.float64)      # (16, 64)
        m0 = a0.max(axis=1)
        v0 = np.exp(a0 - m0[:, None]).astype(np.float32)
        w0 = np.zeros((128, B_LOC), dtype=np.float32)
        for b in range(B_LOC):
            h = b % 2
            w0[h * 64:(h + 1) * 64, b] = v0[b]
        m0_all.append(m0)

        # layout permutation: xs[l, 64h+f, 64i+t] = sh[l, 2i+h, f, t]
        xs = np.ascontiguousarray(
            sh.reshape(l_steps, NTILES, 2, T, T)
              .transpose(0, 2, 3, 1, 4)
              .reshape(l_steps, 128, 512))

        # gather offsets: element (l, b, tf, tt) lives at
        # ((l*128 + (b%2)*64 + tf) * 512 + (b//2)*64 + tt)
        ll, bb = np.meshgrid(np.arange(l_steps), np.arange(B_LOC),
                             indexing="ij")
        tf, tt = np.divmod(tgt.astype(np.int64), T)
        h = bb % 2
        i = bb // 2
        flat = ((ll * 128 + h * 64 + tf) * 512 + i * 64 + tt).astype(np.int32)
        wgt = msk.astype(np.float32)
        n_gcols = l_steps * B_LOC // 128
        goff = flat.reshape(-1).reshape(n_gcols, 128).T.copy()  # [128, cols]
        gwgt = wgt.reshape(-1).reshape(n_gcols, 128).T.copy()

        in_maps.append({"xs": xs, "w0": w0, "goff": goff,
                        "gwgt": np.ascontiguousarray(gwgt)})
    return in_maps, m0_all


def host_epilogue(results, m0_all):
    """Combine per-core outputs into the scalar loss."""
    part_sum = 0.0
    tg_sum = 0.0
    for core in range(N_CORES):
        res = results[core]
        s = np.asarray(res["out_s"], dtype=np.float64)    # [64, 16] col=batch
        mx = np.asarray(res["out_mx"], dtype=np.float64).reshape(-1, B_LOC)
        tg = np.asarray(res["out_tg"], dtype=np.float64)
        m0 = m0_all[core]
        part_sum += (m0 + np.log(mx).sum(axis=0) + np.log(s[END_TAG])).sum()
        tg_sum += tg.sum()
    return np.float32((part_sum - tg_sum) / B)


_NC_CACHE = {}


def kernel_with_results(scores, target, mask, **run_kwargs):
    l_steps = scores.shape[0]
    if l_steps not in _NC_CACHE:
        _NC_CACHE[l_steps] = build_nc(l_steps)
    nc = _NC_CACHE[l_steps]
    in_maps, m0_all = host_prepare(scores, target, mask, l_steps)
    r = run_bass_kernel_spmd(nc, in_maps, core_ids=list(range(N_CORES)),
                             **run_kwargs)
    return host_epilogue(r.results, m0_all), r


def kernel(scores, target, mask):
    return kernel_with_results(scores, target, mask)[0]


if __name__ == "__main__":
    import jax
    key = jax.random.key(0)
    import jax.numpy as jnp
    k1, k2 = jax.random.split(key)
    scores = np.asarray(jax.random.normal(k1, (L, B, T, T), dtype=jnp.float32))
    target = np.asarray(jax.random.randint(k2, (L, B, 1), 0, T * T,
                                           dtype=jnp.int64))
    mask = np.ones((L, B), dtype=bool)
    print(kernel(scores, target, mask))
